# revision 55
# baseline (speedup 1.0000x reference)
"""Trainium2 Bass kernel for CausalSelfAttention with kron-structured bias and
column-masked causal attention.

Shapes (hardcoded): x (4,1024,512), H=8 heads, HD=64, attn_bias (8,64,64)
expanded by kron(ones(8,8)) onto the top-left 512x512 of the (1024,1024)
score matrix. Causal tril mask with every 16th column (j%16==15) zeroed.

Sharding: 8 cores = 4 batches x 2 head-groups (4 heads each). Every core runs
an identical program on its own slice.

v2 design (vs fp32r baseline):
  - all matmul operands bf16 (halves DMA bytes, removes the fp32r 4x penalty
    on <256-col matmuls, 2x DVE throughput on elementwise ops),
  - inputs packed into a few large HBM tensors, issued across BOTH hwdge
    queues (sync + scalar) so the ~600ns/issue serialization disappears,
  - the kron bias is folded into the score matmul: per-head K^T and Q^T are
    packed with ET (one-hot block expansion) / BMQ (repeated bias) in
    partitions 64..127, so one 128-deep matmul computes K.Q + ET.BMQ; the
    bias vanishes outside the 512x512 region because ET is zero for keys>=512
    and BMQ is zero for queries>=512,
  - V bias+ones column added by a fused vector op instead of a matmul,
  - softmax reciprocal via vector reciprocal_approx_fast ([1,512], ~5x
    faster than the 3.35us InstReciprocal),
  - Z partials written bf16 and summed on host.
"""

import sys
import types

import numpy as np
import ml_dtypes

import concourse.bass as bass
import concourse.bacc as bacc
import concourse.tile as tile
from concourse import mybir
from concourse.bass_utils import run_bass_kernel_spmd


def _ensure_axon_hooks():
    """bass_utils' trace path imports antenv.axon_hooks unconditionally; some
    images lack that module. Provide it (and register the real NTFF hook when
    the axon boot shim is available) so tracing degrades gracefully."""
    try:
        import antenv.axon_hooks  # noqa: F401
        return
    except ImportError:
        pass
    m = types.ModuleType("antenv.axon_hooks")
    m._hook = None
    m.set_axon_ntff_profile_hook = lambda h: setattr(m, "_hook", h)
    m.get_axon_ntff_profile_hook = lambda: m._hook
    sys.modules["antenv.axon_hooks"] = m
    try:
        import antenv
        antenv.axon_hooks = m
    except ImportError:
        pass
    try:
        from trn_agent_boot.trn_boot import _ntff_profile_via_ctypes
        m.set_axon_ntff_profile_hook(
            _ntff_profile_via_ctypes("/opt/axon/libaxon_pjrt.so")
        )
    except Exception:
        pass


_ensure_axon_hooks()

F32 = mybir.dt.float32
F32R = mybir.dt.float32r
BF16 = mybir.dt.bfloat16
AFT = mybir.ActivationFunctionType
ALU = mybir.AluOpType
BF = ml_dtypes.bfloat16

B, T, C, H = 4, 1024, 512, 8
HD = 64
SCALE = 1.0 / 8.0
GH = 4          # heads per core
N_CORES = 8

_CACHE = {}
LAST_RESULTS = None


def _kernel_body(tc, io, stage=99):
    nc = tc.nc

    from contextlib import ExitStack
    with ExitStack() as ctx:
        const = ctx.enter_context(tc.tile_pool(name="const", bufs=1))
        pmm = ctx.enter_context(tc.tile_pool(name="pmm", bufs=2, space="PSUM"))
        ps = ctx.enter_context(tc.tile_pool(name="ps", bufs=3, space="PSUM"))
        pot = ctx.enter_context(tc.tile_pool(name="pot", bufs=2, space="PSUM"))
        pbc = ctx.enter_context(tc.tile_pool(name="pbc", bufs=1, space="PSUM"))
        spt = ctx.enter_context(tc.tile_pool(name="spt", bufs=12))
        sden = ctx.enter_context(tc.tile_pool(name="sden", bufs=3))
        szout = ctx.enter_context(tc.tile_pool(name="szout", bufs=2))

        def ctile(shape, tag, dt=BF16):
            return const.tile(shape, dt, tag=tag, name=tag)

        xt = [ctile([128, T], f"xt{i}") for i in range(4)]
        wqk = [ctile([128, 1024], f"wqk{i}") for i in range(2)]
        vw = ctile([128, 1040], "vw")
        vb = ctile([128, 260], "vb")
        keall = ctile([128, 4096], "keall")
        qball = ctile([128, 4096], "qball")
        wpt = ctile([128, 1024], "wpt")
        tri = ctile([128, 128], "tri")
        cmask = ctile([128, 1], "cmask", F32)
        bqbk = ctile([128, 4], "bqbk", F32)
        ones1 = ctile([1, 64], "ones1", BF16)
        ve = [ctile([128, 260], f"ve{i}") for i in range(8)]
        yt = [ctile([128, T], f"yt{i}") for i in range(2)]

        # ---- input DMAs: all on the sync hwdge queue, in priority order, so
        # the DMA engines drain the critical tensors (wqk, x) first. KER is
        # shipped once and replicated to the other heads by SBUF-SBUF DMAs;
        # QBR ships only the nonzero 512-col blocks (zeros via memset).
        nc.sync.dma_start(out=bqbk, in_=io["BQBK"][:, :])
        nc.sync.dma_start(out=wqk[0], in_=io["WQK"][0])
        for i in range(4):
            nc.sync.dma_start(out=xt[i], in_=io["XT"][i])
        nc.sync.dma_start(out=wqk[1], in_=io["WQK"][1])
        nc.sync.dma_start(out=keall[64:128, 0:1024], in_=io["KER"][:, :])
        for hp in range(4):
            nc.sync.dma_start(
                out=qball[64:128, hp * 1024:hp * 1024 + 512], in_=io["QBR"][hp]
            )
            nc.vector.memset(
                qball[64:128, hp * 1024 + 512:(hp + 1) * 1024], 0.0
            )
        nc.sync.dma_start(out=vw, in_=io["VW"][:, :])
        nc.sync.dma_start(out=tri, in_=io["TRI"][:, :])
        nc.sync.dma_start(out=cmask, in_=io["CMASK"][:, :])
        nc.sync.dma_start(out=vb, in_=io["VB"][:, :])
        nc.sync.dma_start(out=ones1, in_=io["ONES1"][:, :])
        nc.sync.dma_start(out=wpt, in_=io["WPT"][:, :])
        for hp in range(1, 4):
            nc.sync.dma_start(
                out=keall[64:128, hp * 1024:(hp + 1) * 1024],
                in_=keall[64:128, 0:1024],
            )

        # ---- K/Q projection group and V-tile emitters. Projections are
        # interleaved into the attention schedule: only what the next
        # attention block needs is emitted ahead of it, so the (scalar-bound)
        # exp stream starts as early as possible.
        # proj 0 = Q (-> qball, bias cols 0..1), proj 1 = K (-> keall, 2..3)
        def qk_group(proj, dt, ib):
            dest = keall if proj == 1 else qball
            mmp = pmm.tile([128, 512], F32, tag="mm", name="mmp")
            for ct in range(4):
                nc.tensor.matmul(
                    mmp,
                    wqk[dt][:, ct * 256 + proj * 128:
                            ct * 256 + (proj + 1) * 128],
                    xt[ct][:, ib * 512:(ib + 1) * 512],
                    start=(ct == 0), stop=(ct == 3),
                )
            for hh in range(2):
                hp = 2 * dt + hh
                dst = dest[0:64, hp * 1024 + ib * 512:
                           hp * 1024 + (ib + 1) * 512]
                src = mmp[hh * 64:(hh + 1) * 64, :]
                bias = bqbk[hh * 64:(hh + 1) * 64,
                            proj * 2 + dt:proj * 2 + dt + 1]
                if proj == 1:
                    nc.scalar.activation(dst, src, AFT.Identity, bias=bias)
                else:
                    nc.vector.tensor_scalar_add(dst, src, bias)

        def v_tile(jt):
            vp = pmm.tile([128, 260], F32, tag="mm", name="vp")
            for ct in range(4):
                nc.tensor.matmul(
                    vp,
                    xt[ct][:, jt * 128:(jt + 1) * 128],
                    vw[:, ct * 260:(ct + 1) * 260],
                    start=(ct == 0), stop=(ct == 3),
                )
            nc.vector.scalar_tensor_tensor(
                ve[jt], vp, 1.0, vb, op0=ALU.mult, op1=ALU.add
            )

        qk_group(1, 0, 0)
        qk_group(0, 0, 0)
        for jt in range(4):
            v_tile(jt)

        if stage <= 1:
            zs1 = szout.tile([128, 512], BF16, tag="z", name="zs1")
            nc.vector.tensor_copy(zs1, qball[:, 1024:1536])
            nc.sync.dma_start(out=io["Z"][:, 0:512], in_=zs1)
            return

        # ---- attention: per (head, query-block); kron bias folded into the
        # 128-deep score matmul via the ET/BMQ rows. Software-pipelined:
        # PV(jt) is emitted after S(jt+1), and the normalization of each
        # block is deferred into the next block's jt loop so the PE never
        # waits on the vector-side reciprocal chain. otp row 0 is the
        # softmax denominator (ones column packed first in ve).
        def finalize(hp, blk, otp):
            def run():
                q0 = blk * 512
                # broadcast the (bf16-rounded) denominator row to 64
                # partitions via PE, then one fused DVE divide
                dentb = sden.tile([1, 512], BF16, tag="dentb", name="dentb")
                nc.vector.tensor_copy(dentb, otp[64:65, :])
                bcp = pbc.tile([64, 512], F32, tag="bc", name="bcp")
                nc.tensor.matmul(bcp, ones1, dentb, start=True, stop=True)
                rec64 = sden.tile([64, 512], F32, tag="rec64", name="rec64")
                with nc.allow_low_precision(reason="softmax denominator"):
                    nc.vector.reciprocal_approx_fast(rec64, bcp)
                nc.vector.tensor_mul(
                    yt[hp // 2][(hp % 2) * 64:(hp % 2) * 64 + 64,
                                q0:q0 + 512],
                    otp[0:64, :], rec64
                )
            return run

        # Block-level software pipelining: block i's PV matmuls (and the
        # interleaved projection/V "filler" matmuls) are spread between block
        # i+1's score matmuls, so the scalar-engine exp stream — the binding
        # resource of the attention phase — is never starved by PE FIFO
        # bursts. pt tiles live one block longer (spt bufs=12).
        def mk_pv(pjt, pc0, ppt, pstart, pstop, potp, php):
            def run():
                nc.tensor.matmul(
                    potp[:, pc0:],
                    ve[pjt][:, 65 * php:65 * php + 65],
                    ppt[:, pc0:],
                    start=pstart, stop=pstop,
                )
            return run

        def qk_fillers(proj, dt, ib):
            hold = {}

            def mk(ct):
                def run():
                    if ct == 0:
                        hold["mmp"] = pmm.tile([128, 512], F32, tag="mm",
                                               name="mmp")
                    mmp = hold["mmp"]
                    nc.tensor.matmul(
                        mmp,
                        wqk[dt][:, ct * 256 + proj * 128:
                                ct * 256 + (proj + 1) * 128],
                        xt[ct][:, ib * 512:(ib + 1) * 512],
                        start=(ct == 0), stop=(ct == 3),
                    )
                    if ct == 3:
                        dest = keall if proj == 1 else qball
                        for hh in range(2):
                            hp = 2 * dt + hh
                            dst = dest[0:64, hp * 1024 + ib * 512:
                                       hp * 1024 + (ib + 1) * 512]
                            src = mmp[hh * 64:(hh + 1) * 64, :]
                            bias = bqbk[hh * 64:(hh + 1) * 64,
                                        proj * 2 + dt:proj * 2 + dt + 1]
                            if proj == 1:
                                nc.scalar.activation(dst, src, AFT.Identity,
                                                     bias=bias)
                            else:
                                nc.vector.tensor_scalar_add(dst, src, bias)
                return run
            return [mk(ct) for ct in range(4)]

        def v_fillers(jt):
            hold = {}

            def mk(ct):
                def run():
                    if ct == 0:
                        hold["vp"] = pmm.tile([128, 260], F32, tag="mm",
                                              name="vp")
                    vp = hold["vp"]
                    nc.tensor.matmul(
                        vp,
                        xt[ct][:, jt * 128:(jt + 1) * 128],
                        vw[:, ct * 260:(ct + 1) * 260],
                        start=(ct == 0), stop=(ct == 3),
                    )
                    if ct == 3:
                        nc.vector.scalar_tensor_tensor(
                            ve[jt], vp, 1.0, vb, op0=ALU.mult, op1=ALU.add
                        )
                return run
            return [mk(ct) for ct in range(4)]

        # Z projection tile emitter: tiles 0..3 ride as late fillers inside
        # the last attention block (they only need blk-0 outputs), tiles
        # 4..7 run after the last finalize.
        def z_tile(it):
            zp = pmm.tile([128, 512], F32, tag="mm", name="zp")
            for ct in range(2):
                nc.tensor.matmul(
                    zp,
                    yt[ct][:, it * 128:(it + 1) * 128],
                    wpt[:, ct * 512:(ct + 1) * 512],
                    start=(ct == 0), stop=(ct == 1),
                )
            zs = szout.tile([128, 512], BF16, tag="z", name="zs")
            if it < 4:
                nc.vector.tensor_copy(zs, zp)
            else:
                nc.scalar.activation(zs, zp, AFT.Copy)
            eng = nc.sync if it % 2 == 0 else nc.scalar
            eng.dma_start(out=io["Z"][:, it * 512:(it + 1) * 512], in_=zs)

        sched = [
            (0, 0, qk_fillers(0, 0, 1), False),
            (0, 1, qk_fillers(1, 0, 1) + v_fillers(4) + v_fillers(5), False),
            (1, 0, v_fillers(6) + v_fillers(7), False),
            (1, 1, qk_fillers(1, 1, 0) + qk_fillers(0, 1, 0), False),
            (2, 0, qk_fillers(0, 1, 1) + qk_fillers(1, 1, 1), False),
            (2, 1, [], False), (3, 0, [], False),
            (3, 1, [(lambda i=i: z_tile(i)) for i in range(4)], True),
        ]
        prev = None       # (otp, pvs, fin)
        for hp, blk, fillers, late in sched:
            h0 = hp * 1024
            q0 = blk * 512
            otp = pot.tile([65, 512], F32, tag="ot", name="otp")
            njt = 4 * (blk + 1)
            pvs = []
            prev_pvs = prev[1] if prev else []
            npv, nf = len(prev_pvs), len(fillers)
            for jt in range(njt):
                m = jt - 4 * blk      # >=0: diagonal-crossing tile
                c0 = 128 * m if m >= 0 else 0
                sp = ps.tile([128, 512], F32, tag="s", name="sp")
                nc.tensor.matmul(
                    sp[:, c0:],
                    keall[:, h0 + jt * 128:h0 + (jt + 1) * 128],
                    qball[:, h0 + q0 + c0:h0 + q0 + 512],
                    start=True, stop=True,
                )
                pt = spt.tile([128, 512], BF16, tag="pt", name="pt")
                nc.scalar.activation(
                    pt[:, c0:], sp[:, c0:], AFT.Exp, bias=cmask[:, 0:1]
                )
                if m >= 0:
                    nc.vector.tensor_mul(
                        pt[:, c0:c0 + 128], pt[:, c0:c0 + 128], tri
                    )
                if stage <= 2:
                    if blk == 0 and hp == 0 and jt == 0:
                        zs2 = szout.tile([128, 512], BF16, tag="z",
                                         name="zs2")
                        nc.vector.tensor_copy(zs2, pt)
                        nc.sync.dma_start(out=io["Z"][:, 0:512], in_=zs2)
                    continue
                pvs.append(mk_pv(jt, c0, pt, jt == 0, jt == njt - 1,
                                 otp, hp))
                if late and jt > 0:
                    # last block: own PVs inline with one-slot lag, so the
                    # PV stream finishes with the exp stream
                    pvs[jt - 1]()
                # fillers first (a slot's prev-PV may read a ve tile the
                # same slot's V filler produces)
                if late:
                    # back-loaded fillers: one per slot in the last nf slots
                    fi = jt - (njt - nf)
                    if 0 <= fi < nf:
                        fillers[fi]()
                else:
                    for k in range(nf * jt // njt, nf * (jt + 1) // njt):
                        fillers[k]()
                # prev-PVs evenly spread; front-loaded in the late-filler
                # block so its finalize precedes the z_tile fillers
                if late:
                    k0, k1 = min(npv, 2 * jt), min(npv, 2 * (jt + 1))
                else:
                    k0, k1 = npv * jt // njt, npv * (jt + 1) // njt
                for k in range(k0, k1):
                    prev_pvs[k]()
                    if k == npv - 1:
                        prev[2]()  # previous block's normalization
            if stage <= 2:
                continue
            prev = (otp, pvs[-1:] if late else pvs, finalize(hp, blk, otp))

        if stage > 2 and prev is not None:
            for pv in prev[1]:
                pv()
        pending = prev[2] if (stage > 2 and prev is not None) else None

        if stage == 2:
            return
        if stage == 3:
            if pending is not None:
                pending()
            zs3 = szout.tile([128, 512], BF16, tag="z", name="zs3")
            nc.vector.tensor_copy(zs3, yt[0][:, 0:512])
            nc.sync.dma_start(out=io["Z"][:, 0:512], in_=zs3)
            return

        # ---- remaining Z tiles (4..7) after the last block's finalize
        if pending is not None:
            pending()
            pending = None
        for it in range(4, 8):
            z_tile(it)


def _build(stage=99):
    nc = bacc.Bacc("TRN2", target_bir_lowering=False, debug=False,
                   num_devices=N_CORES)
    io = {}

    def din(name, shape, dt=BF16):
        io[name] = nc.dram_tensor(name, shape, dt, kind="ExternalInput").ap()

    din("XT", (4, 128, T))
    din("WQK", (2, 128, 1024))
    din("VW", (128, 1040))
    din("VB", (128, 260))
    din("KER", (64, 1024))
    din("QBR", (4, 64, 512))
    din("WPT", (128, 1024))
    din("TRI", (128, 128))
    din("CMASK", (128, 1), F32)
    din("BQBK", (128, 4), F32)
    din("ONES1", (1, 64))
    io["Z"] = nc.dram_tensor("Z", (128, 4096), BF16, kind="ExternalOutput").ap()

    with tile.TileContext(nc) as tc:
        _kernel_body(tc, io, stage)
    nc.compile()
    return nc


def _host_prep(x, attn_bias, Wq, bq, Wk, bk, Wv, bv, Wp, bp):
    """Build the 8 per-core input maps."""
    f = np.float32

    # ET pattern (key-block one-hot) / TRI / CMASK are core-independent
    KER = np.zeros((64, 1024), f)
    for gj in range(64):
        KER[gj, gj * 8:(gj + 1) * 8] = 1.0
    TRI = (np.arange(128)[None, :] >= np.arange(128)[:, None]).astype(f)
    CMASK = np.zeros((128, 1), f)
    CMASK[15::16] = -1e30
    ONES1 = np.ones((1, 64), f)

    in_maps = []
    for core in range(N_CORES):
        b, g = core // 2, core % 2
        gs = slice(256 * g, 256 * (g + 1))

        XT = np.ascontiguousarray(
            x[b].T.reshape(4, 128, T), dtype=f).astype(BF)

        WQK = np.zeros((2, 128, 1024), f)
        WqT = (Wq[gs, :] * SCALE).T      # (512, 256)
        WkT = Wk[gs, :].T                # (512, 256)
        for dt in range(2):
            ds = slice(128 * dt, 128 * (dt + 1))
            for ct in range(4):
                rs = slice(128 * ct, 128 * (ct + 1))
                WQK[dt][:, ct * 256:ct * 256 + 128] = WqT[rs, ds]
                WQK[dt][:, ct * 256 + 128:(ct + 1) * 256] = WkT[rs, ds]

        # ones column last (-> otp row 64 = softmax denominator)
        VW = np.zeros((128, 1040), f)
        VB = np.zeros((128, 260), f)
        for hp in range(GH):
            r = slice(256 * g + 64 * hp, 256 * g + 64 * hp + 64)
            WvT = Wv[r, :].T             # (512, 64)
            for ct in range(4):
                VW[:, ct * 260 + 65 * hp:ct * 260 + 65 * hp + 64] = \
                    WvT[128 * ct:128 * (ct + 1), :]
            VB[:, 65 * hp:65 * hp + 64] = bv[r][None, :]
            VB[:, 65 * hp + 64] = 1.0

        QBR = np.zeros((4, 64, 512), f)
        for hp in range(GH):
            h = GH * g + hp
            QBR[hp] = np.repeat(attn_bias[h], 8, axis=0).T

        WPT = np.zeros((128, 1024), f)
        for ct in range(2):
            r = slice(256 * g + 128 * ct, 256 * g + 128 * (ct + 1))
            WPT[:, ct * 512:(ct + 1) * 512] = Wp[:, r].T

        BQBK = np.zeros((128, 4), f)
        BQBK[:, 0] = (bq[gs] * SCALE)[0:128]
        BQBK[:, 1] = (bq[gs] * SCALE)[128:256]
        BQBK[:, 2] = bk[gs][0:128]
        BQBK[:, 3] = bk[gs][128:256]

        in_maps.append({
            "XT": XT,
            "WQK": WQK.astype(BF),
            "VW": VW.astype(BF),
            "VB": VB.astype(BF),
            "KER": KER.astype(BF),
            "QBR": QBR.astype(BF),
            "WPT": WPT.astype(BF),
            "TRI": TRI.astype(BF),
            "CMASK": CMASK,
            "BQBK": BQBK,
            "ONES1": ONES1.astype(BF),
        })
    return in_maps


def kernel(**inputs):
    global LAST_RESULTS
    if "nc" not in _CACHE:
        _CACHE["nc"] = _build()
    nc = _CACHE["nc"]

    in_maps = _host_prep(**{k: np.asarray(v) for k, v in inputs.items()})
    res = run_bass_kernel_spmd(nc, in_maps, core_ids=list(range(N_CORES)))
    LAST_RESULTS = res

    bp = np.asarray(inputs["bp"], np.float32)
    out = np.empty((B, T, C), np.float32)
    for b in range(B):
        z0 = np.asarray(res.results[2 * b]["Z"], np.float32)
        z1 = np.asarray(res.results[2 * b + 1]["Z"], np.float32)
        z = (z0 + z1).reshape(128, 8, 512).transpose(1, 0, 2).reshape(T, C)
        out[b] = z + bp[None, :]
    return out


# revision 56
# speedup vs baseline: 1.0212x; 1.0212x over previous
"""Trainium2 Bass kernel for CausalSelfAttention with kron-structured bias and
column-masked causal attention.

Shapes (hardcoded): x (4,1024,512), H=8 heads, HD=64, attn_bias (8,64,64)
expanded by kron(ones(8,8)) onto the top-left 512x512 of the (1024,1024)
score matrix. Causal tril mask with every 16th column (j%16==15) zeroed.

Sharding: 8 cores = 4 batches x 2 head-groups (4 heads each). Every core runs
an identical program on its own slice.

v2 design (vs fp32r baseline):
  - all matmul operands bf16 (halves DMA bytes, removes the fp32r 4x penalty
    on <256-col matmuls, 2x DVE throughput on elementwise ops),
  - inputs packed into a few large HBM tensors, issued across BOTH hwdge
    queues (sync + scalar) so the ~600ns/issue serialization disappears,
  - the kron bias is folded into the score matmul: per-head K^T and Q^T are
    packed with ET (one-hot block expansion) / BMQ (repeated bias) in
    partitions 64..127, so one 128-deep matmul computes K.Q + ET.BMQ; the
    bias vanishes outside the 512x512 region because ET is zero for keys>=512
    and BMQ is zero for queries>=512,
  - V bias+ones column added by a fused vector op instead of a matmul,
  - softmax reciprocal via vector reciprocal_approx_fast ([1,512], ~5x
    faster than the 3.35us InstReciprocal),
  - Z partials written bf16 and summed on host.
"""

import sys
import types

import numpy as np
import ml_dtypes

import concourse.bass as bass
import concourse.bacc as bacc
import concourse.tile as tile
from concourse import mybir
from concourse.bass_utils import run_bass_kernel_spmd


def _ensure_axon_hooks():
    """bass_utils' trace path imports antenv.axon_hooks unconditionally; some
    images lack that module. Provide it (and register the real NTFF hook when
    the axon boot shim is available) so tracing degrades gracefully."""
    try:
        import antenv.axon_hooks  # noqa: F401
        return
    except ImportError:
        pass
    m = types.ModuleType("antenv.axon_hooks")
    m._hook = None
    m.set_axon_ntff_profile_hook = lambda h: setattr(m, "_hook", h)
    m.get_axon_ntff_profile_hook = lambda: m._hook
    sys.modules["antenv.axon_hooks"] = m
    try:
        import antenv
        antenv.axon_hooks = m
    except ImportError:
        pass
    try:
        from trn_agent_boot.trn_boot import _ntff_profile_via_ctypes
        m.set_axon_ntff_profile_hook(
            _ntff_profile_via_ctypes("/opt/axon/libaxon_pjrt.so")
        )
    except Exception:
        pass


_ensure_axon_hooks()

F32 = mybir.dt.float32
F32R = mybir.dt.float32r
BF16 = mybir.dt.bfloat16
AFT = mybir.ActivationFunctionType
ALU = mybir.AluOpType
BF = ml_dtypes.bfloat16

B, T, C, H = 4, 1024, 512, 8
HD = 64
SCALE = 1.0 / 8.0
GH = 4          # heads per core
N_CORES = 8

_CACHE = {}
LAST_RESULTS = None


def _kernel_body(tc, io, stage=99):
    nc = tc.nc

    from contextlib import ExitStack
    with ExitStack() as ctx:
        const = ctx.enter_context(tc.tile_pool(name="const", bufs=1))
        pmm = ctx.enter_context(tc.tile_pool(name="pmm", bufs=2, space="PSUM"))
        ps = ctx.enter_context(tc.tile_pool(name="ps", bufs=3, space="PSUM"))
        pot = ctx.enter_context(tc.tile_pool(name="pot", bufs=2, space="PSUM"))
        pbc = ctx.enter_context(tc.tile_pool(name="pbc", bufs=1, space="PSUM"))
        spt = ctx.enter_context(tc.tile_pool(name="spt", bufs=12))
        sden = ctx.enter_context(tc.tile_pool(name="sden", bufs=3))
        szout = ctx.enter_context(tc.tile_pool(name="szout", bufs=2))

        def ctile(shape, tag, dt=BF16):
            return const.tile(shape, dt, tag=tag, name=tag)

        xt = [ctile([128, T], f"xt{i}") for i in range(4)]
        wqk = [ctile([128, 1024], f"wqk{i}") for i in range(2)]
        vw = ctile([128, 1040], "vw")
        vb = ctile([128, 260], "vb")
        keall = ctile([128, 4096], "keall")
        qball = ctile([128, 4096], "qball")
        wpt = ctile([128, 1024], "wpt")
        tri = ctile([128, 128], "tri")
        cmask = ctile([128, 1], "cmask", F32)
        bqbk = ctile([128, 4], "bqbk", F32)
        ones1 = ctile([1, 64], "ones1", BF16)
        ve = [ctile([128, 260], f"ve{i}") for i in range(8)]
        yt = [ctile([128, T], f"yt{i}") for i in range(2)]

        # ---- input DMAs: all on the sync hwdge queue, in priority order, so
        # the DMA engines drain the critical tensors (wqk, x) first. KER is
        # shipped once and replicated to the other heads by SBUF-SBUF DMAs;
        # QBR ships only the nonzero 512-col blocks (zeros via memset).
        nc.sync.dma_start(out=bqbk, in_=io["BQBK"][:, :])
        nc.sync.dma_start(out=wqk[0], in_=io["WQK"][0])
        for i in range(4):
            nc.sync.dma_start(out=xt[i], in_=io["XT"][i])
        nc.sync.dma_start(out=wqk[1], in_=io["WQK"][1])
        nc.sync.dma_start(out=keall[64:128, 0:1024], in_=io["KER"][:, :])
        for hp in range(4):
            nc.sync.dma_start(
                out=qball[64:128, hp * 1024:hp * 1024 + 512], in_=io["QBR"][hp]
            )
            nc.vector.memset(
                qball[64:128, hp * 1024 + 512:(hp + 1) * 1024], 0.0
            )
        nc.sync.dma_start(out=vw, in_=io["VW"][:, :])
        nc.sync.dma_start(out=tri, in_=io["TRI"][:, :])
        nc.sync.dma_start(out=cmask, in_=io["CMASK"][:, :])
        nc.sync.dma_start(out=vb, in_=io["VB"][:, :])
        nc.sync.dma_start(out=ones1, in_=io["ONES1"][:, :])
        nc.sync.dma_start(out=wpt, in_=io["WPT"][:, :])
        for hp in range(1, 4):
            nc.sync.dma_start(
                out=keall[64:128, hp * 1024:(hp + 1) * 1024],
                in_=keall[64:128, 0:1024],
            )

        # ---- K/Q projection group and V-tile emitters. Projections are
        # interleaved into the attention schedule: only what the next
        # attention block needs is emitted ahead of it, so the (scalar-bound)
        # exp stream starts as early as possible.
        # proj 0 = Q (-> qball, bias cols 0..1), proj 1 = K (-> keall, 2..3)
        def qk_group(proj, dt, ib):
            # pre-attention groups: both adds on vector (scalar must stay
            # clear for the first exps; vector is idle during the load)
            dest = keall if proj == 1 else qball
            mmp = pmm.tile([128, 512], F32, tag="mm", name="mmp")
            for ct in range(4):
                nc.tensor.matmul(
                    mmp,
                    wqk[dt][:, ct * 256 + proj * 128:
                            ct * 256 + (proj + 1) * 128],
                    xt[ct][:, ib * 512:(ib + 1) * 512],
                    start=(ct == 0), stop=(ct == 3),
                )
            for hh in range(2):
                hp = 2 * dt + hh
                nc.vector.tensor_scalar_add(
                    dest[0:64, hp * 1024 + ib * 512:
                         hp * 1024 + (ib + 1) * 512],
                    mmp[hh * 64:(hh + 1) * 64, :],
                    bqbk[hh * 64:(hh + 1) * 64,
                         proj * 2 + dt:proj * 2 + dt + 1],
                )

        def v_tile(jt):
            vp = pmm.tile([128, 260], F32, tag="mm", name="vp")
            for ct in range(4):
                nc.tensor.matmul(
                    vp,
                    xt[ct][:, jt * 128:(jt + 1) * 128],
                    vw[:, ct * 260:(ct + 1) * 260],
                    start=(ct == 0), stop=(ct == 3),
                )
            nc.vector.scalar_tensor_tensor(
                ve[jt], vp, 1.0, vb, op0=ALU.mult, op1=ALU.add
            )

        qk_group(1, 0, 0)
        qk_group(0, 0, 0)
        for jt in range(4):
            v_tile(jt)

        if stage <= 1:
            zs1 = szout.tile([128, 512], BF16, tag="z", name="zs1")
            nc.vector.tensor_copy(zs1, qball[:, 1024:1536])
            nc.sync.dma_start(out=io["Z"][:, 0:512], in_=zs1)
            return

        # ---- attention: per (head, query-block); kron bias folded into the
        # 128-deep score matmul via the ET/BMQ rows. Software-pipelined:
        # PV(jt) is emitted after S(jt+1), and the normalization of each
        # block is deferred into the next block's jt loop so the PE never
        # waits on the vector-side reciprocal chain. otp row 0 is the
        # softmax denominator (ones column packed first in ve).
        def finalize(hp, blk, otp):
            def run():
                q0 = blk * 512
                # broadcast the (bf16-rounded) denominator row to 64
                # partitions via PE, then one fused DVE divide
                dentb = sden.tile([1, 512], BF16, tag="dentb", name="dentb")
                nc.vector.tensor_copy(dentb, otp[64:65, :])
                bcp = pbc.tile([64, 512], F32, tag="bc", name="bcp")
                nc.tensor.matmul(bcp, ones1, dentb, start=True, stop=True)
                rec64 = sden.tile([64, 512], F32, tag="rec64", name="rec64")
                with nc.allow_low_precision(reason="softmax denominator"):
                    nc.vector.reciprocal_approx_fast(rec64, bcp)
                nc.vector.tensor_mul(
                    yt[hp // 2][(hp % 2) * 64:(hp % 2) * 64 + 64,
                                q0:q0 + 512],
                    otp[0:64, :], rec64
                )
            return run

        # Block-level software pipelining: block i's PV matmuls (and the
        # interleaved projection/V "filler" matmuls) are spread between block
        # i+1's score matmuls, so the scalar-engine exp stream — the binding
        # resource of the attention phase — is never starved by PE FIFO
        # bursts. pt tiles live one block longer (spt bufs=12).
        def mk_pv(pjt, pc0, ppt, pstart, pstop, potp, php):
            def run():
                nc.tensor.matmul(
                    potp[:, pc0:],
                    ve[pjt][:, 65 * php:65 * php + 65],
                    ppt[:, pc0:],
                    start=pstart, stop=pstop,
                )
            return run

        def qk_fillers(proj, dt, ib):
            hold = {}

            def mk(ct):
                def run():
                    if ct == 0:
                        hold["mmp"] = pmm.tile([128, 512], F32, tag="mm",
                                               name="mmp")
                    mmp = hold["mmp"]
                    nc.tensor.matmul(
                        mmp,
                        wqk[dt][:, ct * 256 + proj * 128:
                                ct * 256 + (proj + 1) * 128],
                        xt[ct][:, ib * 512:(ib + 1) * 512],
                        start=(ct == 0), stop=(ct == 3),
                    )
                    if ct == 3:
                        dest = keall if proj == 1 else qball
                        for hh in range(2):
                            hp = 2 * dt + hh
                            dst = dest[0:64, hp * 1024 + ib * 512:
                                       hp * 1024 + (ib + 1) * 512]
                            src = mmp[hh * 64:(hh + 1) * 64, :]
                            bias = bqbk[hh * 64:(hh + 1) * 64,
                                        proj * 2 + dt:proj * 2 + dt + 1]
                            if proj == 1:
                                nc.scalar.activation(dst, src, AFT.Identity,
                                                     bias=bias)
                            else:
                                nc.vector.tensor_scalar_add(dst, src, bias)
                return run
            return [mk(ct) for ct in range(4)]

        def v_fillers(jt):
            hold = {}

            def mk(ct):
                def run():
                    if ct == 0:
                        hold["vp"] = pmm.tile([128, 260], F32, tag="mm",
                                              name="vp")
                    vp = hold["vp"]
                    nc.tensor.matmul(
                        vp,
                        xt[ct][:, jt * 128:(jt + 1) * 128],
                        vw[:, ct * 260:(ct + 1) * 260],
                        start=(ct == 0), stop=(ct == 3),
                    )
                    if ct == 3:
                        nc.vector.scalar_tensor_tensor(
                            ve[jt], vp, 1.0, vb, op0=ALU.mult, op1=ALU.add
                        )
                return run
            return [mk(ct) for ct in range(4)]

        # Z projection tile emitter: tiles 0..3 ride as late fillers inside
        # the last attention block (they only need blk-0 outputs), tiles
        # 4..7 run after the last finalize.
        def z_tile(it):
            zp = pmm.tile([128, 512], F32, tag="mm", name="zp")
            for ct in range(2):
                nc.tensor.matmul(
                    zp,
                    yt[ct][:, it * 128:(it + 1) * 128],
                    wpt[:, ct * 512:(ct + 1) * 512],
                    start=(ct == 0), stop=(ct == 1),
                )
            zs = szout.tile([128, 512], BF16, tag="z", name="zs")
            if it < 4:
                nc.vector.tensor_copy(zs, zp)
            else:
                nc.scalar.activation(zs, zp, AFT.Copy)
            eng = nc.sync if it % 2 == 0 else nc.scalar
            eng.dma_start(out=io["Z"][:, it * 512:(it + 1) * 512], in_=zs)

        sched = [
            (0, 0, qk_fillers(0, 0, 1), False),
            (0, 1, qk_fillers(1, 0, 1) + v_fillers(4) + v_fillers(5), False),
            (1, 0, v_fillers(6) + v_fillers(7), False),
            (1, 1, qk_fillers(1, 1, 0) + qk_fillers(0, 1, 0), False),
            (2, 0, qk_fillers(0, 1, 1) + qk_fillers(1, 1, 1), False),
            (2, 1, [], False), (3, 0, [], False),
            (3, 1, [(lambda i=i: z_tile(i)) for i in range(4)], True),
        ]
        prev = None       # (otp, pvs, fin)
        for hp, blk, fillers, late in sched:
            h0 = hp * 1024
            q0 = blk * 512
            otp = pot.tile([65, 512], F32, tag="ot", name="otp")
            njt = 4 * (blk + 1)
            pvs = []
            prev_pvs = prev[1] if prev else []
            npv, nf = len(prev_pvs), len(fillers)
            for jt in range(njt):
                m = jt - 4 * blk      # >=0: diagonal-crossing tile
                c0 = 128 * m if m >= 0 else 0
                sp = ps.tile([128, 512], F32, tag="s", name="sp")
                nc.tensor.matmul(
                    sp[:, c0:],
                    keall[:, h0 + jt * 128:h0 + (jt + 1) * 128],
                    qball[:, h0 + q0 + c0:h0 + q0 + 512],
                    start=True, stop=True,
                )
                pt = spt.tile([128, 512], BF16, tag="pt", name="pt")
                nc.scalar.activation(
                    pt[:, c0:], sp[:, c0:], AFT.Exp, bias=cmask[:, 0:1]
                )
                if m >= 0:
                    nc.vector.tensor_mul(
                        pt[:, c0:c0 + 128], pt[:, c0:c0 + 128], tri
                    )
                if stage <= 2:
                    if blk == 0 and hp == 0 and jt == 0:
                        zs2 = szout.tile([128, 512], BF16, tag="z",
                                         name="zs2")
                        nc.vector.tensor_copy(zs2, pt)
                        nc.sync.dma_start(out=io["Z"][:, 0:512], in_=zs2)
                    continue
                pvs.append(mk_pv(jt, c0, pt, jt == 0, jt == njt - 1,
                                 otp, hp))
                if late and jt > 0:
                    # last block: own PVs inline with one-slot lag, so the
                    # PV stream finishes with the exp stream
                    pvs[jt - 1]()
                # fillers first (a slot's prev-PV may read a ve tile the
                # same slot's V filler produces)
                if late:
                    # back-loaded fillers: one per slot in the last nf slots
                    fi = jt - (njt - nf)
                    if 0 <= fi < nf:
                        fillers[fi]()
                else:
                    for k in range(nf * jt // njt, nf * (jt + 1) // njt):
                        fillers[k]()
                # prev-PVs evenly spread; front-loaded in the late-filler
                # block so its finalize precedes the z_tile fillers
                if late:
                    k0, k1 = min(npv, 2 * jt), min(npv, 2 * (jt + 1))
                else:
                    k0, k1 = npv * jt // njt, npv * (jt + 1) // njt
                for k in range(k0, k1):
                    prev_pvs[k]()
                    if k == npv - 1:
                        prev[2]()  # previous block's normalization
            if stage <= 2:
                continue
            prev = (otp, pvs[-1:] if late else pvs, finalize(hp, blk, otp))

        if stage > 2 and prev is not None:
            for pv in prev[1]:
                pv()
        pending = prev[2] if (stage > 2 and prev is not None) else None

        if stage == 2:
            return
        if stage == 3:
            if pending is not None:
                pending()
            zs3 = szout.tile([128, 512], BF16, tag="z", name="zs3")
            nc.vector.tensor_copy(zs3, yt[0][:, 0:512])
            nc.sync.dma_start(out=io["Z"][:, 0:512], in_=zs3)
            return

        # ---- remaining Z tiles (4..7) after the last block's finalize
        if pending is not None:
            pending()
            pending = None
        for it in range(4, 8):
            z_tile(it)


def _build(stage=99):
    nc = bacc.Bacc("TRN2", target_bir_lowering=False, debug=False,
                   num_devices=N_CORES)
    io = {}

    def din(name, shape, dt=BF16):
        io[name] = nc.dram_tensor(name, shape, dt, kind="ExternalInput").ap()

    din("XT", (4, 128, T))
    din("WQK", (2, 128, 1024))
    din("VW", (128, 1040))
    din("VB", (128, 260))
    din("KER", (64, 1024))
    din("QBR", (4, 64, 512))
    din("WPT", (128, 1024))
    din("TRI", (128, 128))
    din("CMASK", (128, 1), F32)
    din("BQBK", (128, 4), F32)
    din("ONES1", (1, 64))
    io["Z"] = nc.dram_tensor("Z", (128, 4096), BF16, kind="ExternalOutput").ap()

    with tile.TileContext(nc) as tc:
        _kernel_body(tc, io, stage)
    nc.compile()
    return nc


def _host_prep(x, attn_bias, Wq, bq, Wk, bk, Wv, bv, Wp, bp):
    """Build the 8 per-core input maps."""
    f = np.float32

    # ET pattern (key-block one-hot) / TRI / CMASK are core-independent
    KER = np.zeros((64, 1024), f)
    for gj in range(64):
        KER[gj, gj * 8:(gj + 1) * 8] = 1.0
    TRI = (np.arange(128)[None, :] >= np.arange(128)[:, None]).astype(f)
    CMASK = np.zeros((128, 1), f)
    CMASK[15::16] = -1e30
    ONES1 = np.ones((1, 64), f)

    in_maps = []
    for core in range(N_CORES):
        b, g = core // 2, core % 2
        gs = slice(256 * g, 256 * (g + 1))

        XT = np.ascontiguousarray(
            x[b].T.reshape(4, 128, T), dtype=f).astype(BF)

        WQK = np.zeros((2, 128, 1024), f)
        WqT = (Wq[gs, :] * SCALE).T      # (512, 256)
        WkT = Wk[gs, :].T                # (512, 256)
        for dt in range(2):
            ds = slice(128 * dt, 128 * (dt + 1))
            for ct in range(4):
                rs = slice(128 * ct, 128 * (ct + 1))
                WQK[dt][:, ct * 256:ct * 256 + 128] = WqT[rs, ds]
                WQK[dt][:, ct * 256 + 128:(ct + 1) * 256] = WkT[rs, ds]

        # ones column last (-> otp row 64 = softmax denominator)
        VW = np.zeros((128, 1040), f)
        VB = np.zeros((128, 260), f)
        for hp in range(GH):
            r = slice(256 * g + 64 * hp, 256 * g + 64 * hp + 64)
            WvT = Wv[r, :].T             # (512, 64)
            for ct in range(4):
                VW[:, ct * 260 + 65 * hp:ct * 260 + 65 * hp + 64] = \
                    WvT[128 * ct:128 * (ct + 1), :]
            VB[:, 65 * hp:65 * hp + 64] = bv[r][None, :]
            VB[:, 65 * hp + 64] = 1.0

        QBR = np.zeros((4, 64, 512), f)
        for hp in range(GH):
            h = GH * g + hp
            QBR[hp] = np.repeat(attn_bias[h], 8, axis=0).T

        WPT = np.zeros((128, 1024), f)
        for ct in range(2):
            r = slice(256 * g + 128 * ct, 256 * g + 128 * (ct + 1))
            WPT[:, ct * 512:(ct + 1) * 512] = Wp[:, r].T

        BQBK = np.zeros((128, 4), f)
        BQBK[:, 0] = (bq[gs] * SCALE)[0:128]
        BQBK[:, 1] = (bq[gs] * SCALE)[128:256]
        BQBK[:, 2] = bk[gs][0:128]
        BQBK[:, 3] = bk[gs][128:256]

        in_maps.append({
            "XT": XT,
            "WQK": WQK.astype(BF),
            "VW": VW.astype(BF),
            "VB": VB.astype(BF),
            "KER": KER.astype(BF),
            "QBR": QBR.astype(BF),
            "WPT": WPT.astype(BF),
            "TRI": TRI.astype(BF),
            "CMASK": CMASK,
            "BQBK": BQBK,
            "ONES1": ONES1.astype(BF),
        })
    return in_maps


def kernel(**inputs):
    global LAST_RESULTS
    if "nc" not in _CACHE:
        _CACHE["nc"] = _build()
    nc = _CACHE["nc"]

    in_maps = _host_prep(**{k: np.asarray(v) for k, v in inputs.items()})
    res = run_bass_kernel_spmd(nc, in_maps, core_ids=list(range(N_CORES)))
    LAST_RESULTS = res

    bp = np.asarray(inputs["bp"], np.float32)
    out = np.empty((B, T, C), np.float32)
    for b in range(B):
        z0 = np.asarray(res.results[2 * b]["Z"], np.float32)
        z1 = np.asarray(res.results[2 * b + 1]["Z"], np.float32)
        z = (z0 + z1).reshape(128, 8, 512).transpose(1, 0, 2).reshape(T, C)
        out[b] = z + bp[None, :]
    return out
